# revision 13
# baseline (speedup 1.0000x reference)
"""Group-wise correlation cost volume (build_gwc_volume) on 8 trn2 cores.

volume[b,g,d,h,w] = sum_c ref[b,g,c,h,w] * tgt[b,g,c,h,w-d]  (0 where w<d)

Sharding: 16 (b,g) pairs across 8 cores, 2 pairs per core. Each pair is a
contiguous 64-channel slice of the inputs and a contiguous [D,H,W] slab of
the output.

Per (b,g,h) the volume rows are diagonals of the Gram matrix
G[w',w] = sum_c tgt[c,w'] * ref[c,w].  Only the band d = w - w' in [0,48)
is needed, so the Gram is computed as 4 column-piece matmuls (M=64,
stationary T[:, 64k:64k+64]), each with a 112-wide moving window
R[:, BASE_k : BASE_k+112) written at a fixed offset of a PSUM bank.
The two (b,g) pairs sit on PE row halves and the 2 column pieces on PE
column halves, so 4 matmuls share the 128x128 array.

M=64 (not 32): the PE instruction stream is fetched from HBM in 16 KiB
IRAM blocks and every block boundary stalls the PE ~3 us.  M=32 needs
4096 PE instructions (16 blocks, ~13 mid-run stalls = ~30 us dead); M=64
needs 2048 (8 blocks), at the cost of a wider band (112 vs 80 per row).

The kernel rides both walls at once (ridge): PE stream ~64 us + stalls,
DMA engines ~75 us (in 16.8 MB + out 14.7 MB over 16 engines at
~26 GB/s).  Structure keeps both HWDGE rings busy end to end:
 - ref+tgt host-stacked into ONE dram tensor: each h-chunk is a single
   2.1 MB in-DMA on the sync ring.
 - band tiles leave as half-chunk out-DMAs on the ACT ring.
 - in-tiles 4-deep for prefetch runway.
PSUM->SBUF copies (f32 src caps both engines at 1x) split between DVE
(tensor_copy) and ACT (activation-copy).

Everything crosses HBM as bf16: inputs cast on the host, the PSUM f32
band cast to bf16 on the PSUM->SBUF copy (2e-2 budget >> ~4e-3 bf16
error).  Two h-rows pack per PSUM bank (2*224 = 448 f32 <= 512).

Diagonal (shear) extraction at 1-partition granularity is not expressible
in any engine's access patterns, so the band tiles are DMAed out and the
diagonals are gathered on the host during unsharding.
"""

import sys

if "/opt/trn_rl_repo" not in sys.path:
    sys.path.insert(0, "/opt/trn_rl_repo")

import ml_dtypes
import numpy as np

import concourse.bacc as bacc
import concourse.tile as tile
from concourse import mybir
from concourse.bass_utils import run_bass_kernel_spmd

F32 = mybir.dt.float32
BF16 = mybir.dt.bfloat16
NP_BF16 = ml_dtypes.bfloat16

B, C, H, W = 2, 512, 128, 256
G, CG, D = 8, 64, 48
N_CORES = 8
PAIRS = 2  # (b,g) pairs per core
HC = 16  # h rows per chunk
M = 64  # w'-rows per piece
PW = M + D - 1 + 1  # piece window width (112)
NP_ = 256 // M  # pieces (4)

# piece k covers w' in [Mk, Mk+M); its moving window starts at
# BASE[k] = min(Mk, W - PW) so every piece is a full PW columns.
BASE = [min(M * k, W - PW) for k in range(NP_)]

_cached = {}


def _build_module():
    nc = bacc.Bacc("TRN2", target_bir_lowering=False, debug=False, num_devices=N_CORES)
    # dim0: 0 = ref, 1 = tgt (host-stacked so one DMA covers both)
    rt = nc.dram_tensor("rt", [2, PAIRS, CG, H, W], BF16, kind="ExternalInput")
    # band tiles, layout [pair, w'-row, h, x]: cols 0:112 pieces 0-1
    # (w' in [0,128)), cols 112:224 pieces 2-3 (w' in [128,256))
    out_bt = nc.dram_tensor(
        "out_bt", [PAIRS, 128, H, 2 * PW], BF16, kind="ExternalOutput"
    )

    rt_p = rt.rearrange("t pr c h w -> t (pr c) h w")

    with tile.TileContext(nc) as tc:
        with (
            tc.tile_pool(name="ins", bufs=4) as ins,
            tc.tile_pool(name="stage", bufs=4) as stage_pool,
            tc.tile_pool(name="psum", bufs=4, space="PSUM") as psum,
        ):
            for ch in range(H // HC):
                h0 = ch * HC
                it = ins.tile([128, 2, HC, W], BF16, tag="it")
                nc.sync.dma_start(
                    it[:], rt_p[:, :, h0 : h0 + HC, :].rearrange("t p h w -> p t h w")
                )
                st = stage_pool.tile(
                    [128, PAIRS, HC, 2 * PW], BF16, tag="st", name=f"st_{ch}"
                )
                banks = {}
                for i, hl0 in enumerate(range(0, HC, 2)):
                    for pr in range(PAIRS):
                        banks[pr] = psum.tile(
                            [128, 2, 2 * PW],
                            F32,
                            tag=f"bk{pr}",
                            name=f"bk{pr}_{ch}_{hl0}",
                        )
                    # pairs innermost: consecutive PE instructions alternate
                    # row groups so LDWEIGHTS pulls ahead of the other row
                    # half's in-flight matmul
                    for hj in range(2):
                        hl = hl0 + hj
                        for k in range(NP_):
                            c0 = PW * (k // 2)
                            m0 = M * (k % 2)
                            for pr in range(PAIRS):
                                p0 = pr * CG
                                nc.tensor.matmul(
                                    banks[pr][m0 : m0 + M, hj, c0 : c0 + PW],
                                    it[p0 : p0 + CG, 1, hl, M * k : M * k + M],
                                    it[p0 : p0 + CG, 0, hl, BASE[k] : BASE[k] + PW],
                                    tile_position=(p0, m0),
                                )
                    for pr in range(PAIRS):
                        # split the PSUM->SBUF drain between DVE and ACT
                        # (different banks => parallel PSUM access is legal)
                        if (i + pr) % 2 == 0:
                            nc.vector.tensor_copy(
                                st[:, pr, hl0 : hl0 + 2, :], banks[pr][:]
                            )
                        else:
                            nc.scalar.copy(st[:, pr, hl0 : hl0 + 2, :], banks[pr][:])
                    # drain each half chunk as soon as its groups land so the
                    # out stream flows early instead of bursting per chunk
                    if hl0 == 6:
                        nc.scalar.dma_start(
                            out_bt[:, :, h0 : h0 + 8, :].rearrange(
                                "pr p h c -> p pr h c"
                            ),
                            st[:, :, 0:8, :],
                        )
                    elif hl0 == 14:
                        nc.scalar.dma_start(
                            out_bt[:, :, h0 + 8 : h0 + HC, :].rearrange(
                                "pr p h c -> p pr h c"
                            ),
                            st[:, :, 8:HC, :],
                        )

    nc.compile()
    return nc


def _get_module():
    if "nc" not in _cached:
        _cached["nc"] = _build_module()
    return _cached["nc"]


def _make_in_maps(refimg_fea, targetimg_fea):
    ref = np.asarray(refimg_fea, dtype=np.float32).astype(NP_BF16)
    tgt = np.asarray(targetimg_fea, dtype=np.float32).astype(NP_BF16)
    assert ref.shape == (B, C, H, W)
    rp = ref.reshape(B * G, CG, H, W)
    tp = tgt.reshape(B * G, CG, H, W)
    return [
        {
            "rt": np.ascontiguousarray(
                np.stack([rp[2 * k : 2 * k + 2], tp[2 * k : 2 * k + 2]])
            )
        }
        for k in range(N_CORES)
    ]


def _host_extract(bt):
    """Gather band diagonals into the full volume.

    bt: [16, 128, H, 224] f32.  Row p holds G[w', w = BASE[k] + x] at col
    112*(k//2) + x where k = w'//64 indexes the piece (pieces 0-1 at cols
    0:112 for w' = row, pieces 2-3 at cols 112:224 for w' = row + 128).
    vol[d,h,w] = G[w-d, w] -> row (w-d) % 128, col from the piece table.
    """
    d = np.arange(D)[:, None]
    w = np.arange(W)[None, :]
    wp = w - d  # [D, W] source w' (negative -> zero region)
    valid = wp >= 0
    wpc = np.clip(wp, 0, None)
    k = wpc // M
    base = np.minimum(M * k, W - PW)
    col = PW * (k // 2) + (w - base)
    row = wpc % 128

    vol = np.zeros((B * G, D, H, W), np.float32)
    for pair in range(B * G):
        t = bt[pair].transpose(1, 0, 2)  # [h, row, col]
        r = t[:, row, col]  # [H, D, W]
        # where() not *=: stray inf/nan garbage x 0 would poison zeros
        r = np.where(valid[None], r, 0.0)
        vol[pair] = r.transpose(1, 0, 2)
    return vol.reshape(B, G, D, H, W)


def kernel(refimg_fea, targetimg_fea, num_groups, maxdisp):
    assert int(num_groups) == G and int(maxdisp) == D

    in_maps = _make_in_maps(refimg_fea, targetimg_fea)
    nc = _get_module()
    res = run_bass_kernel_spmd(nc, in_maps, core_ids=list(range(N_CORES)))

    bt = np.concatenate(
        [np.asarray(r["out_bt"]).astype(np.float32) for r in res.results], axis=0
    )
    return _host_extract(bt)


# revision 14
# speedup vs baseline: 1.1046x; 1.1046x over previous
"""Group-wise correlation cost volume (build_gwc_volume) on 8 trn2 cores.

volume[b,g,d,h,w] = sum_c ref[b,g,c,h,w] * tgt[b,g,c,h,w-d]  (0 where w<d)

Sharding: 16 (b,g) pairs across 8 cores, 2 pairs per core. Each pair is a
contiguous 64-channel slice of the inputs and a contiguous [D,H,W] slab of
the output.

Per (b,g,h) the volume rows are diagonals of the Gram matrix
G[w',w] = sum_c tgt[c,w'] * ref[c,w].  Only the band d = w - w' in [0,48)
is needed, so the Gram is computed as 4 column-piece matmuls (M=64,
stationary T[:, 64k:64k+64]), each with a 112-wide moving window
R[:, BASE_k : BASE_k+112) written at a fixed offset of a PSUM bank.
The two (b,g) pairs sit on PE row halves and the 2 column pieces on PE
column halves, so 4 matmuls share the 128x128 array.

M=64 (not 32): the PE instruction stream is fetched from HBM in 16 KiB
IRAM blocks and every block boundary stalls the PE ~3 us.  M=32 needs
4096 PE instructions (16 blocks, ~13 mid-run stalls = ~30 us dead); M=64
needs 2048 (8 blocks), at the cost of a wider band (112 vs 80 per row).

The kernel rides both walls at once (ridge): PE stream ~64 us + stalls,
DMA engines ~75 us (in 16.8 MB + out 14.7 MB over 16 engines at
~26 GB/s).  Structure keeps both HWDGE rings busy end to end:
 - ref+tgt host-stacked into ONE dram tensor: each h-chunk is a single
   2.1 MB in-DMA on the sync ring.
 - band tiles leave as half-chunk out-DMAs on the ACT ring.
 - in-tiles 4-deep for prefetch runway.
PSUM->SBUF copies (f32 src caps both engines at 1x) split between DVE
(tensor_copy) and ACT (activation-copy).

Everything crosses HBM as bf16: inputs cast on the host, the PSUM f32
band cast to bf16 on the PSUM->SBUF copy (2e-2 budget >> ~4e-3 bf16
error).  Two h-rows pack per PSUM bank (2*224 = 448 f32 <= 512).

Diagonal (shear) extraction at 1-partition granularity is not expressible
in any engine's access patterns, so the band tiles are DMAed out and the
diagonals are gathered on the host during unsharding.
"""

import sys

if "/opt/trn_rl_repo" not in sys.path:
    sys.path.insert(0, "/opt/trn_rl_repo")

import ml_dtypes
import numpy as np

import concourse.bacc as bacc
import concourse.tile as tile
from concourse import mybir
from concourse.bass_utils import run_bass_kernel_spmd

F32 = mybir.dt.float32
BF16 = mybir.dt.bfloat16
NP_BF16 = ml_dtypes.bfloat16

B, C, H, W = 2, 512, 128, 256
G, CG, D = 8, 64, 48
N_CORES = 8
PAIRS = 2  # (b,g) pairs per core
HC = 16  # h rows per chunk
M = 64  # w'-rows per piece
PW = M + D - 1 + 1  # piece window width (112)
NP_ = 256 // M  # pieces (4)

# piece k covers w' in [Mk, Mk+M); its moving window starts at
# BASE[k] = min(Mk, W - PW) so every piece is a full PW columns.
BASE = [min(M * k, W - PW) for k in range(NP_)]

_cached = {}


def _build_module():
    nc = bacc.Bacc("TRN2", target_bir_lowering=False, debug=False, num_devices=N_CORES)
    # dim0: 0 = ref, 1 = tgt (host-stacked so one DMA covers both)
    rt = nc.dram_tensor("rt", [2, PAIRS, CG, H, W], BF16, kind="ExternalInput")
    # band tiles, layout [pair, w'-row, h, x]: cols 0:112 pieces 0-1
    # (w' in [0,128)), cols 112:224 pieces 2-3 (w' in [128,256))
    out_bt = nc.dram_tensor(
        "out_bt", [PAIRS, 128, H, 2 * PW], BF16, kind="ExternalOutput"
    )

    rt_p = rt.rearrange("t pr c h w -> t (pr c) h w")

    with tile.TileContext(nc) as tc:
        with (
            tc.tile_pool(name="ins", bufs=4) as ins,
            tc.tile_pool(name="stage", bufs=4) as stage_pool,
            tc.tile_pool(name="psum", bufs=4, space="PSUM") as psum,
        ):
            for ch in range(H // HC):
                h0 = ch * HC
                it = ins.tile([128, 2, HC, W], BF16, tag="it")
                nc.sync.dma_start(
                    it[:], rt_p[:, :, h0 : h0 + HC, :].rearrange("t p h w -> p t h w")
                )
                st = stage_pool.tile(
                    [128, PAIRS, HC, 2 * PW], BF16, tag="st", name=f"st_{ch}"
                )
                banks = {}
                for i, hl0 in enumerate(range(0, HC, 2)):
                    for pr in range(PAIRS):
                        banks[pr] = psum.tile(
                            [128, 2, 2 * PW],
                            F32,
                            tag=f"bk{pr}",
                            name=f"bk{pr}_{ch}_{hl0}",
                        )
                    # pairs innermost: consecutive PE instructions alternate
                    # row groups so LDWEIGHTS pulls ahead of the other row
                    # half's in-flight matmul
                    for hj in range(2):
                        hl = hl0 + hj
                        for k in range(NP_):
                            c0 = PW * (k // 2)
                            m0 = M * (k % 2)
                            for pr in range(PAIRS):
                                p0 = pr * CG
                                nc.tensor.matmul(
                                    banks[pr][m0 : m0 + M, hj, c0 : c0 + PW],
                                    it[p0 : p0 + CG, 1, hl, M * k : M * k + M],
                                    it[p0 : p0 + CG, 0, hl, BASE[k] : BASE[k] + PW],
                                    tile_position=(p0, m0),
                                )
                    for pr in range(PAIRS):
                        # split the PSUM->SBUF drain between DVE and ACT
                        # (different banks => parallel PSUM access is legal)
                        if (i + pr) % 2 == 0:
                            nc.vector.tensor_copy(
                                st[:, pr, hl0 : hl0 + 2, :], banks[pr][:]
                            )
                        else:
                            nc.scalar.copy(st[:, pr, hl0 : hl0 + 2, :], banks[pr][:])
                    # drain each half chunk as soon as its groups land so the
                    # out stream flows early instead of bursting per chunk.
                    # Issued from GPSIMD (SWDGE): a dma_start waiting on DVE
                    # copies would head-of-line-block ACT's own copy queue,
                    # starving the PE of free PSUM banks.
                    if hl0 == 6:
                        nc.gpsimd.dma_start(
                            out_bt[:, :, h0 : h0 + 8, :].rearrange(
                                "pr p h c -> p pr h c"
                            ),
                            st[:, :, 0:8, :],
                        )
                    elif hl0 == 14:
                        nc.gpsimd.dma_start(
                            out_bt[:, :, h0 + 8 : h0 + HC, :].rearrange(
                                "pr p h c -> p pr h c"
                            ),
                            st[:, :, 8:HC, :],
                        )

    nc.compile()
    return nc


def _get_module():
    if "nc" not in _cached:
        _cached["nc"] = _build_module()
    return _cached["nc"]


def _make_in_maps(refimg_fea, targetimg_fea):
    ref = np.asarray(refimg_fea, dtype=np.float32).astype(NP_BF16)
    tgt = np.asarray(targetimg_fea, dtype=np.float32).astype(NP_BF16)
    assert ref.shape == (B, C, H, W)
    rp = ref.reshape(B * G, CG, H, W)
    tp = tgt.reshape(B * G, CG, H, W)
    return [
        {
            "rt": np.ascontiguousarray(
                np.stack([rp[2 * k : 2 * k + 2], tp[2 * k : 2 * k + 2]])
            )
        }
        for k in range(N_CORES)
    ]


def _host_extract(bt):
    """Gather band diagonals into the full volume.

    bt: [16, 128, H, 224] f32.  Row p holds G[w', w = BASE[k] + x] at col
    112*(k//2) + x where k = w'//64 indexes the piece (pieces 0-1 at cols
    0:112 for w' = row, pieces 2-3 at cols 112:224 for w' = row + 128).
    vol[d,h,w] = G[w-d, w] -> row (w-d) % 128, col from the piece table.
    """
    d = np.arange(D)[:, None]
    w = np.arange(W)[None, :]
    wp = w - d  # [D, W] source w' (negative -> zero region)
    valid = wp >= 0
    wpc = np.clip(wp, 0, None)
    k = wpc // M
    base = np.minimum(M * k, W - PW)
    col = PW * (k // 2) + (w - base)
    row = wpc % 128

    vol = np.zeros((B * G, D, H, W), np.float32)
    for pair in range(B * G):
        t = bt[pair].transpose(1, 0, 2)  # [h, row, col]
        r = t[:, row, col]  # [H, D, W]
        # where() not *=: stray inf/nan garbage x 0 would poison zeros
        r = np.where(valid[None], r, 0.0)
        vol[pair] = r.transpose(1, 0, 2)
    return vol.reshape(B, G, D, H, W)


def kernel(refimg_fea, targetimg_fea, num_groups, maxdisp):
    assert int(num_groups) == G and int(maxdisp) == D

    in_maps = _make_in_maps(refimg_fea, targetimg_fea)
    nc = _get_module()
    res = run_bass_kernel_spmd(nc, in_maps, core_ids=list(range(N_CORES)))

    bt = np.concatenate(
        [np.asarray(r["out_bt"]).astype(np.float32) for r in res.results], axis=0
    )
    return _host_extract(bt)
